# revision 22
# baseline (speedup 1.0000x reference)
"""Trainium2 Bass kernel v3 for nn_LSTMModel: single fused NEFF.

Layer-pipelined across core pairs: even core 2g runs LSTM layer 1 for batch
group g (16 rows), odd core 2g+1 runs layer 2 for the same rows, lagging two
32-step blocks. h1 blocks move between pair cores via a per-block 2-core
AllGather whose output lands DIRECTLY in the unified "reg" gather region
(no DRAM bounce). Every core fetches its per-block GEMM input by an
indirect DMA whose row offsets are per-core *data* — keeping the SPMD
program identical across cores.

v3 vs v2:
- The input-projection GEMM for block s+1 (64 MMs free=512) is interleaved
  into steps 8..31 of block s instead of running as a serial blob at
  superstep start; each unit's PSUM->SBUF copies are deferred one step so
  the Act-queued copy never gates the in-order PE via the PSUM ring.
- Embedding staging for block s+2 (indirect gather + PE transpose + f8
  cast) is likewise interleaved; block zero-pads are written once in the
  prefix.
- Per step the two hidden-chunk halves (A: j 0,1 / B: j 2,3) live in
  separate full-bank PSUM tiles (deps are bank-granular), the xw fold is
  the PSUM-initializing write (start=True, off the matmul->sigmoid path),
  and each half's sigmoid/elementwise chain runs as soon as its last U
  pass stops, overlapping the other half and the next step's first kc
  passes.
- h state is kept in split A/B SBUF tensors; global max pool is folded
  incrementally per step on DVE (one step deferred), removing the
  per-block TensorReduce blob.

Head (pool AllGather over 8 cores, dense+relu, vocab-sharded softmax with
AllReduce of partial sums) unchanged. All matmuls bf16/fp8 with fp32 PSUM.
Biases are all zero (asserted).
"""

import numpy as np
import ml_dtypes

import concourse.bass as bass
import concourse.bacc as bacc
import concourse.mybir as mybir
import concourse.tile as tile
from concourse.masks import make_identity

bf16 = mybir.dt.bfloat16
f32 = mybir.dt.float32
i32 = mybir.dt.int32
f8 = mybir.dt.float8e4
AF = mybir.ActivationFunctionType
ALU = mybir.AluOpType
bf = ml_dtypes.bfloat16

B, V, D, M = 64, 50000, 128, 512
NC = 8
BGRP = 4                 # batch groups (core pairs)
BL = B // BGRP           # 16 batch rows per pair
VS = V // NC             # 6250 vocab cols per core
SB = 32                  # steps per block
KC = M // 128            # 4 hidden chunks
MC = 4 * M // 128        # 16 gate chunks
FILL_FROM = 8            # first step index that takes interleaved fill work

T = None
NBLK = NSUP = NTOK = NGATH = GPB = BW = ZROW = CCROW = NREG = None


def configure(t):
    """Set sequence length T and derived constants (module globals)."""
    global T, NBLK, NSUP, NTOK, NGATH, GPB, BW, ZROW, CCROW, NREG
    T = t
    NBLK = T // SB           # blocks
    NSUP = NBLK + 2          # supersteps (odd cores lag by 2)
    NTOK = BL * T            # tokens per group
    NGATH = NTOK // 128      # gather tiles
    GPB = NGATH // NBLK      # 4 gather tiles per block
    BW = KC * SB * BL        # 2048: cols of one block buffer [128, kc, s, b]
    ZROW = NBLK * 128        # reg row base of the zero block
    CCROW = (NBLK + 1) * 128  # reg row base of CC landing (2 parities x 256)
    NREG = CCROW + 2 * 256


configure(512)


def _new_nc():
    return bacc.Bacc("TRN2", target_bir_lowering=False, debug=False,
                     num_devices=NC)


def build_fused(reps=1, no_cc=False):
    NCH = (VS + 511) // 512
    nc = _new_nc()
    ids_d = nc.dram_tensor("ids", [128, NGATH], i32, kind="ExternalInput")
    emb_d = nc.dram_tensor("emb", [V, D], f32, kind="ExternalInput")
    goff_d = nc.dram_tensor("goff", [128, NSUP], i32, kind="ExternalInput")
    u_d = nc.dram_tensor("ut", [128, KC * MC * 128], f8, kind="ExternalInput")
    w_d = nc.dram_tensor("wt", [128, KC * MC * 128], f8, kind="ExternalInput")
    wd_d = nc.dram_tensor("wdt", [128, KC * KC * 128], bf16, kind="ExternalInput")
    wo_d = nc.dram_tensor("wot", [128, KC * VS], bf16, kind="ExternalInput")
    pr_d = nc.dram_tensor("probs", [B, VS], f32, kind="ExternalOutput")

    reg = nc.dram_tensor("reg", [NREG, BW], f8, kind="Internal")
    cin_d = nc.dram_tensor("cc_in", [2 * 128, BW], f8, kind="Internal")
    pin_d = nc.dram_tensor("p_in", [128, KC * BL], f32, kind="Internal")
    pout_d = nc.dram_tensor("p_all", [NC * 128, KC * BL], f32, kind="Internal")
    sin_d = nc.dram_tensor("s_in", [B, 1], f32, kind="Internal")
    sout_d = nc.dram_tensor("s_all", [B, 1], f32, kind="Internal")

    pair_groups = [[2 * g, 2 * g + 1] for g in range(BGRP)]
    all_groups = [list(range(NC))]

    with tile.TileContext(nc) as tc:
        with tc.tile_pool(name="wts", bufs=1) as wpool, \
             tc.tile_pool(name="sb", bufs=3) as pool, \
             tc.tile_pool(name="st", bufs=2) as spool, \
             tc.tile_pool(name="zps", bufs=1, space="PSUM") as psp, \
             tc.tile_pool(name="gps", bufs=2, space="PSUM") as gpsp, \
             tc.tile_pool(name="stp", bufs=1, space="PSUM") as stp, \
             tc.tile_pool(name="pst", bufs=1, space="PSUM") as pst:
          for _rep in range(reps):
            u = wpool.tile([128, KC * MC * 128], f8, tag="u")
            w = wpool.tile([128, KC * MC * 128], f8, tag="w")
            wd = wpool.tile([128, KC * KC * 128], bf16, tag="wd")
            histA = [wpool.tile([128, BW // 2], f8, tag=f"histA{i}",
                                name=f"histA{i}") for i in range(2)]
            histB = [wpool.tile([128, BW // 2], f8, tag=f"histB{i}",
                                name=f"histB{i}") for i in range(2)]
            xw = [wpool.tile([128, MC * SB * BL], bf16, tag=f"xw{i}",
                             name=f"xw{i}") for i in range(2)]
            gemin = [wpool.tile([128, BW], f8, tag=f"gemin{i}",
                                name=f"gemin{i}") for i in range(2)]
            cst = wpool.tile([128, KC * BL], f32, tag="cst")
            maxp = wpool.tile([128, KC * BL], f32, tag="maxp")
            zero = wpool.tile([128, BW], f8, tag="zero")
            ident = wpool.tile([128, 128], f32, tag="ident")
            ids_t = wpool.tile([128, NGATH], i32, tag="ids")
            goff_t = wpool.tile([128, NSUP], i32, tag="goff")

            nc.sync.dma_start(u[:], u_d[:])
            nc.sync.dma_start(w[:], w_d[:])
            nc.sync.dma_start(wd[:], wd_d[:])
            nc.sync.dma_start(ids_t[:], ids_d[:])
            nc.sync.dma_start(goff_t[:], goff_d[:])
            nc.vector.memset(cst[:], 0.0)
            nc.vector.memset(maxp[:], 0.0)
            nc.vector.memset(zero[:], 0.0)
            nc.vector.memset(histA[1][:], 0.0)   # h_{-1} = 0 slot
            nc.vector.memset(histB[1][:], 0.0)
            make_identity(nc, ident[:])
            ident_b = wpool.tile([128, 128], bf16, tag='identb')
            nc.scalar.copy(ident_b[:], ident[:])

            # reg zero block + zero-pad of kc chunks 1..3 of every eT block
            nc.sync.dma_start(reg[ZROW:ZROW + 128, :], zero[:])
            for j in range(NBLK):
                nc.sync.dma_start(
                    reg[j * 128:(j + 1) * 128, SB * BL:BW],
                    zero[:, 0:BW - SB * BL])

            def stage_tile(j, t):
                """Gather+transpose eT tile t of block j into reg."""
                et = pool.tile([128, 128], f32, tag="gath")
                g = j * GPB + t
                nc.gpsimd.indirect_dma_start(
                    out=et[:], out_offset=None, in_=emb_d[:],
                    in_offset=bass.IndirectOffsetOnAxis(
                        ap=ids_t[:, g:g + 1], axis=0))
                tp = stp.tile([128, 128], f32, tag="tp")
                nc.tensor.transpose(out=tp[:], in_=et[:], identity=ident[:])
                sg = pool.tile([128, 128], f8, tag="sg")
                nc.vector.tensor_scalar(out=sg[:], in0=tp[:], scalar1=16.0,
                                        scalar2=None, op0=ALU.mult)
                nc.sync.dma_start(
                    reg[j * 128:(j + 1) * 128, t * 128:(t + 1) * 128], sg[:])

            def stage_closures(j):
                return [(lambda jj=j, tt=t: stage_tile(jj, tt))
                        for t in range(GPB)]

            for j in (0, 1):
                for f in stage_closures(j):
                    f()

            histA_v = [h[:].rearrange("p (j s b) -> p j s b", j=2, s=SB)
                       for h in histA]
            histB_v = [h[:].rearrange("p (j s b) -> p j s b", j=2, s=SB)
                       for h in histB]

            def hist_half(gi, kc):
                return (histA_v[gi][:, kc, :, :] if kc < 2
                        else histB_v[gi][:, kc - 2, :, :])
            xw_v = [x[:].rearrange("p (j g s b) -> p j g s b", j=KC, g=4, s=SB)
                    for x in xw]
            gem_v = [x[:].rearrange("p (j s b) -> p j s b", j=KC, s=SB)
                     for x in gemin]
            cst_v = cst[:].rearrange("p (j b) -> p j b", j=KC)
            maxp_v = maxp[:].rearrange("p (j b) -> p j b", j=KC)

            def gather(s):
                nc.gpsimd.indirect_dma_start(
                    out=gemin[s % 2][:], out_offset=None, in_=reg[:, :],
                    in_offset=bass.IndirectOffsetOnAxis(
                        ap=goff_t[:, s:s + 1], axis=0))

            gather(0)

            def gemm_units(s):
                """(mm_closures, cp_closures) per mc chunk. The cps are
                emitted one step after the mms so the Act-queued PSUM->SBUF
                copy never gates the in-order PE via the gpsp ring."""
                gi = s % 2
                units = []
                state = {}
                for mc in range(MC):
                    def mk(mc):
                        def mm(kc):
                            def f():
                                if kc == 0:
                                    state[mc] = gpsp.tile(
                                        [128, SB * BL], f32, tag="gemm",
                                        name=f"gp_{mc}")
                                nc.tensor.matmul(
                                    state[mc][:],
                                    w[:, (kc * MC + mc) * 128:
                                      (kc * MC + mc + 1) * 128],
                                    gem_v[gi][:, kc, :, :],
                                    start=(kc == 0), stop=(kc == KC - 1))
                            return f
                        def cp(h0, h1):
                            def f():
                                nc.scalar.copy(
                                    xw_v[gi][:, mc % KC, mc // KC, h0:h1, :],
                                    state[mc][:].rearrange(
                                        "p (s b) -> p s b", s=SB)[:, h0:h1, :])
                            return f
                        return ([mm(kc) for kc in range(KC)],
                                [cp(0, SB // 2), cp(SB // 2, SB)])
                    units.append(mk(mc))
                return units

            A_MCS = [g * KC + j for j in (0, 1) for g in range(4)]
            B_MCS = [g * KC + j for j in (2, 3) for g in range(4)]

            def epilogue(s, st, zph_v, j0, j1):
                gi = s % 2
                jn = j1 - j0
                sig = spool.tile([128, 3 * jn * BL], f32, tag=f"sig{j0}")
                sig_v = sig[:].rearrange("p (j g b) -> p j g b", j=jn, g=3)
                nc.scalar.activation(sig_v[:, :, :, :],
                                     zph_v[:, :, 0:3, :], AF.Sigmoid,
                                     scale=1.0 / 256.0)
                ig = spool.tile([128, jn * BL], f32, tag=f"ig{j0}")
                ig_v = ig[:].rearrange("p (j b) -> p j b", j=jn)
                nc.vector.scalar_tensor_tensor(
                    out=ig_v[:, :, :], in0=zph_v[:, :, 3, :], scalar=0.0,
                    in1=sig_v[:, :, 0, :], op0=ALU.max, op1=ALU.mult)
                fc = spool.tile([128, jn * BL], f32, tag=f"fc{j0}")
                fc_v = fc[:].rearrange("p (j b) -> p j b", j=jn)
                nc.gpsimd.tensor_tensor(
                    out=fc_v[:, :, :], in0=sig_v[:, :, 1, :],
                    in1=cst_v[:, j0:j1, :], op=ALU.mult)
                nc.vector.scalar_tensor_tensor(
                    out=cst_v[:, j0:j1, :], in0=ig_v[:, :, :],
                    scalar=1.0 / 16.0, in1=fc_v[:, :, :],
                    op0=ALU.mult, op1=ALU.add)
                hv = histA_v[gi] if j0 == 0 else histB_v[gi]
                nc.vector.scalar_tensor_tensor(
                    out=hv[:, :, st, :],
                    in0=cst_v[:, j0:j1, :], scalar=0.0,
                    in1=sig_v[:, :, 2, :], op0=ALU.max, op1=ALU.mult)

            def fold_max(s, st):
                """maxp = max(maxp, h_st) on DVE (f8 reads ok there)."""
                gi = s % 2
                nc.vector.tensor_tensor(
                    out=maxp_v[:, 0:2, :], in0=maxp_v[:, 0:2, :],
                    in1=histA_v[gi][:, :, st, :], op=ALU.max)
                nc.vector.tensor_tensor(
                    out=maxp_v[:, 2:4, :], in0=maxp_v[:, 2:4, :],
                    in1=histB_v[gi][:, :, st, :], op=ALU.max)

            def lstm_step(s, st, fill, pre_cps):
                gi = s % 2
                if st == 0:
                    hpp, hrow = 1 - gi, SB - 1
                else:
                    hpp, hrow = gi, st - 1
                for f in pre_cps:
                    f()
                if st > 0:
                    fold_max(s, st - 1)   # deferred: h of st-1 is stable
                # A/B halves in separate (padded full-bank) PSUM tiles so the
                # bank-granular tracker doesn't serialize half B's fold
                # behind half A's epilogue reads.
                zph = [psp.tile([128, 2 * 4 * BL], f32, tag=f"zp{h}",
                                padded_shape=[128, 512], name=f"zp{h}")
                       for h in range(2)]
                zph_v = [z[:].rearrange("p (j g b) -> p j g b", j=2, g=4)
                         for z in zph]
                # xw folds FIRST (start=True): off the matmul->sigmoid path,
                # and both share one ident_b weight load.
                for (hi_, (j0, j1)) in ((0, (0, 2)), (1, (2, 4))):
                    nc.tensor.matmul(
                        zph[hi_][:, 0:128], ident_b[:],
                        xw_v[gi][:, j0:j1, :, st, :], start=True, stop=False)
                for kc in range(KC - 1):
                    for mc in range(MC):
                        g, j = mc // KC, mc % KC
                        nc.tensor.matmul(
                            zph_v[j // 2][:, j % 2, g, :],
                            u[:, (kc * MC + mc) * 128:(kc * MC + mc + 1) * 128],
                            hist_half(hpp, kc)[:, hrow, :],
                            start=False, stop=False)
                kc = KC - 1
                for (hi_, mcs, (j0, j1)) in ((0, A_MCS, (0, 2)),
                                             (1, B_MCS, (2, 4))):
                    for mc in mcs:
                        g, j = mc // KC, mc % KC
                        nc.tensor.matmul(
                            zph_v[hi_][:, j % 2, g, :],
                            u[:, (kc * MC + mc) * 128:(kc * MC + mc + 1) * 128],
                            hist_half(hpp, kc)[:, hrow, :],
                            start=False, stop=True)
                    epilogue(s, st, zph_v[hi_], j0, j1)
                for f in fill:
                    f()

            for mms0, cps0 in gemm_units(0):
                for f in mms0:
                    f()
                for f in cps0:
                    f()

            for s in range(NSUP):
                gi = s % 2
                groups = []
                if s + 1 < NSUP:
                    gather(s + 1)
                    groups.extend(gemm_units(s + 1))
                if s + 2 < NBLK:
                    groups.extend([([f], []) for f in stage_closures(s + 2)])
                ngrp = len(groups)
                span = SB - FILL_FROM
                pend_cps = []
                for st in range(SB):
                    if st < FILL_FROM or ngrp == 0:
                        sl = []
                    else:
                        lo = (st - FILL_FROM) * ngrp // span
                        hi = (st - FILL_FROM + 1) * ngrp // span
                        sl = groups[lo:hi]
                    mms = [f for g_ in sl for f in g_[0]]
                    lstm_step(s, st, mms, pend_cps)
                    pend_cps = [f for g_ in sl for f in g_[1]]
                for f in pend_cps:
                    f()
                fold_max(s, SB - 1)
                if s < NBLK:
                    nc.sync.dma_start(
                        cin_d[gi * 128:(gi + 1) * 128, 0:BW // 2],
                        histA[gi][:])
                    nc.sync.dma_start(
                        cin_d[gi * 128:(gi + 1) * 128, BW // 2:BW],
                        histB[gi][:])
                    land = reg[CCROW + gi * 256:CCROW + (gi + 1) * 256, :]
                    if no_cc:
                        nc.sync.dma_start(
                            land[0:128, :], cin_d[gi * 128:(gi + 1) * 128, :])
                        nc.sync.dma_start(
                            land[128:256, :], cin_d[gi * 128:(gi + 1) * 128, :])
                    else:
                        nc.gpsimd.collective_compute(
                            "AllGather", ALU.bypass, replica_groups=pair_groups,
                            ins=[cin_d[gi * 128:(gi + 1) * 128, :]],
                            outs=[land])

            # ---- head ----
            nc.sync.dma_start(pin_d[:, :], maxp[:])
            if no_cc:
                for _c in range(NC):
                    nc.sync.dma_start(pout_d[_c * 128:(_c + 1) * 128, :],
                                      pin_d[:, :])
            else:
                nc.gpsimd.collective_compute(
                    "AllGather", ALU.bypass, replica_groups=all_groups,
                    ins=[pin_d[:, :]], outs=[pout_d[:, :]])
            pTf = wpool.tile([128, KC * B], f32, tag="pTf")
            pTf_v = pTf[:].rearrange("p (j b) -> p j b", j=KC)
            for g in range(BGRP):
                c_odd = 2 * g + 1
                nc.sync.dma_start(
                    pTf_v[:, :, g * BL:(g + 1) * BL],
                    pout_d[c_odd * 128:(c_odd + 1) * 128, :].rearrange(
                        "p (j b) -> p j b", j=KC))
            pT = wpool.tile([128, KC * B], bf16, tag="pT")
            nc.scalar.activation(pT[:], pTf[:], AF.Copy, scale=1.0 / 16.0)

            dps = pst.tile([128, KC * B], f32, tag="dps")
            for mc in range(KC):
                for kc in range(KC):
                    nc.tensor.matmul(
                        dps[:, mc * B:(mc + 1) * B],
                        wd[:, (kc * KC + mc) * 128:(kc * KC + mc + 1) * 128],
                        pT[:, kc * B:(kc + 1) * B],
                        start=(kc == 0), stop=(kc == KC - 1))
            dT = wpool.tile([128, KC * B], bf16, tag="dT")
            nc.scalar.activation(dT[:], dps[:], AF.Relu)

            expl = wpool.tile([B, VS], f32, tag="expl")
            acc = wpool.tile([B, NCH], f32, tag="acc")
            for ch in range(NCH):
                n0 = ch * 512
                nw = min(512, VS - n0)
                wo_c = pool.tile([128, KC * 512], bf16, tag="wo_c")
                wo_cv = wo_c[:].rearrange("p (j n) -> p j n", j=KC)
                for kc in range(KC):
                    nc.sync.dma_start(wo_cv[:, kc, 0:nw],
                                      wo_d[:, kc * VS + n0:kc * VS + n0 + nw])
                lp = pst.tile([B, 512], f32, tag="lp")
                for kc in range(KC):
                    nc.tensor.matmul(
                        lp[:, 0:nw], dT[:, kc * B:(kc + 1) * B],
                        wo_cv[:, kc, 0:nw],
                        start=(kc == 0), stop=(kc == KC - 1))
                nc.scalar.activation(expl[:, n0:n0 + nw], lp[:, 0:nw], AF.Exp,
                                     accum_out=acc[:, ch:ch + 1])
            sums = pool.tile([B, 1], f32, tag="sums")
            nc.vector.tensor_reduce(sums[:], acc[:], axis=mybir.AxisListType.X,
                                    op=ALU.add)
            nc.sync.dma_start(sin_d[:, :], sums[:])
            if no_cc:
                nc.sync.dma_start(sout_d[:, :], sin_d[:, :])
            else:
                nc.gpsimd.collective_compute(
                    "AllReduce", ALU.add, replica_groups=all_groups,
                    ins=[sin_d[:, :]], outs=[sout_d[:, :]])
            tsum = pool.tile([B, 1], f32, tag="tsum")
            nc.sync.dma_start(tsum[:], sout_d[:, :])
            inv = pool.tile([B, 1], f32, tag="inv")
            nc.vector.reciprocal(inv[:], tsum[:])
            prob = wpool.tile([B, VS], f32, tag="prob")
            nc.vector.tensor_scalar_mul(prob[:], expl[:], inv[:])
            nc.sync.dma_start(pr_d[:, :], prob[:])
    nc.finalize()
    return nc


# --------------------------------------------------------------------------
# host prep
# --------------------------------------------------------------------------

def _perm_gates(w):
    i, f, g, o = np.split(w, 4, axis=-1)
    return np.concatenate([i, f, o, g], axis=-1)


def _tile_lhsT(w, dt=bf):
    K, G = w.shape
    kc, mc = K // 128, G // 128
    return np.ascontiguousarray(
        w.reshape(kc, 128, mc, 128).transpose(1, 0, 2, 3).reshape(128, kc * mc * 128)
    ).astype(dt)


def _prep_ids(x_grp):
    # token order (block j, step s, batch b): ids[lane, tile]
    m = x_grp.reshape(BL, NBLK, SB).transpose(1, 2, 0).reshape(-1)
    return np.ascontiguousarray(m.reshape(NGATH, 128).T).astype(np.int32)


def _goff(is_odd):
    lanes = np.arange(128, dtype=np.int32).reshape(128, 1)
    cols = []
    for s in range(NSUP):
        if not is_odd:
            base = s * 128 if s < NBLK else ZROW
        else:
            base = ZROW if s < 2 else CCROW + (s % 2) * 256
        cols.append(base + lanes)
    return np.concatenate(cols, axis=1)


# --------------------------------------------------------------------------
# cached PJRT runner
# --------------------------------------------------------------------------

def _make_runner(nc):
    import jax
    from jax.experimental.shard_map import shard_map
    from jax.sharding import Mesh, PartitionSpec
    from concourse import bass2jax

    bass2jax.install_neuronx_cc_hook()

    in_names, out_names, out_avals = [], [], []
    partition_name = nc.partition_id_tensor.name if nc.partition_id_tensor else None
    for alloc in nc.m.functions[0].allocations:
        if not isinstance(alloc, mybir.MemoryLocationSet):
            continue
        name = alloc.memorylocations[0].name
        if alloc.kind == "ExternalInput":
            if name != partition_name:
                in_names.append(name)
        elif alloc.kind == "ExternalOutput":
            out_names.append(name)
            out_avals.append(jax.core.ShapedArray(tuple(alloc.tensor_shape),
                                                  mybir.dt.np(alloc.dtype)))
    n_params = len(in_names)
    n_outs = len(out_avals)
    all_in_names = list(in_names) + list(out_names)
    if partition_name is not None:
        all_in_names.append(partition_name)
    donate = tuple(range(n_params, n_params + n_outs))

    def _body(*args):
        operands = list(args)
        if partition_name is not None:
            operands.append(bass2jax.partition_id_tensor())
        outs = bass2jax._bass_exec_p.bind(
            *operands,
            out_avals=tuple(out_avals),
            in_names=tuple(all_in_names),
            out_names=tuple(out_names),
            lowering_input_output_aliases=(),
            sim_require_finite=True,
            sim_require_nnan=True,
            nc=nc,
        )
        return tuple(outs)

    devices = jax.devices()[:NC]
    mesh = Mesh(np.asarray(devices), ("core",))
    in_specs = (PartitionSpec("core"),) * (n_params + n_outs)
    out_specs = (PartitionSpec("core"),) * n_outs
    sharded = jax.jit(
        shard_map(_body, mesh=mesh, in_specs=in_specs, out_specs=out_specs,
                  check_rep=False),
        donate_argnums=donate, keep_unused=True)

    def run(in_maps):
        concat_in = [np.concatenate([np.asarray(m[n]) for m in in_maps], axis=0)
                     for n in in_names]
        zeros = [np.zeros((NC * a.shape[0], *a.shape[1:]), a.dtype)
                 for a in out_avals]
        out_arrs = sharded(*concat_in, *zeros)
        return [
            {n: np.asarray(out_arrs[i]).reshape(NC, *out_avals[i].shape)[c]
             for i, n in enumerate(out_names)}
            for c in range(NC)
        ]

    return run


_CACHE = {}


def _prep_in_maps(x, emb, W1, U1, W2, U2, Wd, Wo):
    f8np = ml_dtypes.float8_e4m3
    w1p = np.concatenate([W1, np.zeros((M - D, 4 * M), np.float32)], axis=0)
    w1t = _tile_lhsT(_perm_gates(w1p) * 16.0, f8np)
    u1t = _tile_lhsT(_perm_gates(U1) * 16.0, f8np)
    w2t = _tile_lhsT(_perm_gates(W2) * 16.0, f8np)
    u2t = _tile_lhsT(_perm_gates(U2) * 16.0, f8np)
    wdt = _tile_lhsT(Wd)
    zero_ids = np.zeros((128, NGATH), np.int32)
    goff_e, goff_o = _goff(False), _goff(True)

    ins = []
    for c in range(NC):
        g = c // 2
        odd = c % 2 == 1
        wos = Wo[:, c * VS:(c + 1) * VS]
        wot = np.ascontiguousarray(
            wos.reshape(KC, 128, VS).transpose(1, 0, 2).reshape(128, KC * VS)
        ).astype(bf)
        ins.append({
            "ids": zero_ids if odd else _prep_ids(x[g * BL:(g + 1) * BL]),
            "emb": emb,
            "goff": goff_o if odd else goff_e,
            "ut": u2t if odd else u1t,
            "wt": w2t if odd else w1t,
            "wdt": wdt,
            "wot": wot,
        })
    return ins


def kernel(x, emb, W1, U1, b1, W2, U2, b2, Wd, bd, Wo, bo):
    x = np.asarray(x)
    assert x.dtype == np.int32
    for b_ in (b1, b2, bd, bo):
        assert not np.asarray(b_).any(), "nonzero biases not supported"

    ins = _prep_in_maps(
        x, np.asarray(emb, np.float32),
        np.asarray(W1, np.float32), np.asarray(U1, np.float32),
        np.asarray(W2, np.float32), np.asarray(U2, np.float32),
        np.asarray(Wd, np.float32), np.asarray(Wo, np.float32))

    key = ("fused", T)
    if key not in _CACHE:
        _CACHE[key] = _make_runner(build_fused())
    res = _CACHE[key](ins)
    probs = np.concatenate([res[c]["probs"] for c in range(NC)], axis=1)
    return probs.astype(np.float32)


def measure_hw_ns(inputs, measure):
    """HW exec time (ns) of the fused kernel, via reps=5 vs reps=1 diff
    (delta-R=4 so the ~±1.5ms min-wall drift contributes <0.4ms)."""
    ins = _prep_in_maps(
        np.asarray(inputs["x"]), np.asarray(inputs["emb"], np.float32),
        np.asarray(inputs["W1"], np.float32), np.asarray(inputs["U1"], np.float32),
        np.asarray(inputs["W2"], np.float32), np.asarray(inputs["U2"], np.float32),
        np.asarray(inputs["Wd"], np.float32), np.asarray(inputs["Wo"], np.float32))
    return measure(build_fused, NC, ins, 1, 5, label="fused")
